# revision 3
# baseline (speedup 1.0000x reference)
"""GAT (3-layer, PPI-style) Bass/Tile kernel for 8 Trainium2 NeuronCores.

Strategy (graph/data parallel, dst-ownership sharding):
  - Nodes are sharded contiguously: core c owns nodes [c*NOWN, (c+1)*NOWN).
  - Edges live on the core owning dst; per core they are grouped by
    128-node dst groups and sorted so that edge-softmax segment reductions
    become dense one-hot matmuls on the tensor engine.
  - Per layer: Phase A computes feat/el/er for owned nodes with one matmul
    against W_aug = [W | W@al_bd | W@ar_bd]; an AllGather publishes
    bf16 [feat, el] rows to every core; batched SWDGE dma_gather fetches
    the per-edge rows (feat+el by src, er by local dst) for BATCH groups
    at a time in 3 large multi-packet calls; exp/leaky-relu run on
    DVE/ACT; one fused one-hot matmul per slot tile accumulates both
    sum_e ex_e * feat_src and sum_e ex_e; node-level normalization, ELU
    and a PE transpose produce the next layer's x^T in bf16.

All graph-dependent index structures are computed on the host inside
kernel() and shipped as tensor inputs, so one SPMD program serves all
8 cores.
"""

import math

import ml_dtypes
import numpy as np

BF16 = ml_dtypes.bfloat16
P = 128
NCORES = 8
BATCH = 4  # dst-node groups per gather batch


# ----------------------------------------------------------------------------
# Host-side preparation
# ----------------------------------------------------------------------------


def _wrap_idxs(idx, k):
    """int16 index array for dma_gather: wrapped in 16 partitions, replicated
    8x across the 128 partitions. idx: [k*128] -> [128, k*8]."""
    assert idx.shape[0] == k * P
    w = idx.astype(np.int16).reshape(k * 8, 16).T  # [16, k*8]
    return np.ascontiguousarray(np.tile(w, (8, 1)))  # [128, k*8]


def _batches(G):
    out = []
    g0 = 0
    while g0 < G:
        out.append(list(range(g0, min(g0 + BATCH, G))))
        g0 += BATCH
    return out


def _prepare(inputs):
    h = np.asarray(inputs["h"], dtype=np.float32)
    src = np.asarray(inputs["src"]).astype(np.int64)
    dst = np.asarray(inputs["dst"]).astype(np.int64)

    N, NFEAT = h.shape
    E = src.shape[0]
    assert N % NCORES == 0
    NOWN = N // NCORES
    G = math.ceil(NOWN / P)
    HALF = (N + 1) // 2
    assert HALF <= 32767, "table half exceeds int16 gather index range"
    assert NOWN <= 32767

    Ws, als, ars = [], [], []
    for i in (1, 2, 3):
        Ws.append(np.asarray(inputs[f"W{i}"], dtype=np.float32))
        als.append(np.asarray(inputs[f"al{i}"], dtype=np.float32))
        ars.append(np.asarray(inputs[f"ar{i}"], dtype=np.float32))
    H = als[0].shape[0]
    FEAT = [W.shape[1] for W in Ws]  # H*D per layer
    D = [f // H for f in FEAT]
    NCLASS = D[-1]

    # W_aug = [W | W @ al_bd | W @ ar_bd] with al_bd[h*D+d, h] = al[h, d]
    Waug = []
    for W, al, ar, f, d in zip(Ws, als, ars, FEAT, D):
        al_bd = np.zeros((f, H), dtype=np.float32)
        ar_bd = np.zeros((f, H), dtype=np.float32)
        for hh in range(H):
            al_bd[hh * d : (hh + 1) * d, hh] = al[hh]
            ar_bd[hh * d : (hh + 1) * d, hh] = ar[hh]
        Waug.append(
            np.concatenate([W, W @ al_bd, W @ ar_bd], axis=1).astype(BF16)
        )
    FO = [f + 2 * H for f in FEAT]
    # bf16 gather-table row widths in elements (bytes multiple of 256)
    ROW = [math.ceil((f + H) * 2 / 256) * 128 for f in FEAT]

    # ---- edge partitioning --------------------------------------------------
    owner = dst // NOWN
    per_core = []
    maxA = maxB = 0
    for c in range(NCORES):
        sel = np.nonzero(owner == c)[0]
        e_src = src[sel]
        e_dst = dst[sel]
        dloc = e_dst - c * NOWN  # 0..NOWN-1
        grp = dloc // P  # dst group
        half = (e_src >= HALF).astype(np.int64)
        order = np.lexsort((e_src, half, grp))
        e_src, dloc, grp, half = e_src[order], dloc[order], grp[order], half[order]
        cntA = np.zeros(G, dtype=np.int64)
        cntB = np.zeros(G, dtype=np.int64)
        for g in range(G):
            m = grp == g
            cntA[g] = int(np.count_nonzero(m & (half == 0)))
            cntB[g] = int(np.count_nonzero(m & (half == 1)))
        per_core.append((e_src, dloc, grp, half, cntA, cntB))
        maxA = max(maxA, int(cntA.max()) if G else 0)
        maxB = max(maxB, int(cntB.max()) if G else 0)

    kA = max(1, math.ceil(maxA / P))
    kB = max(1, math.ceil(maxB / P))
    K = kA + kB
    batches = _batches(G)

    in_maps = []
    for c in range(NCORES):
        e_src, dloc, grp, half, cntA, cntB = per_core[c]
        # per-group slot arrays
        gidxA = np.zeros((G, kA * P), dtype=np.int64)
        gidxB = np.zeros((G, kB * P), dtype=np.int64)
        gdstA = np.full((G, kA * P), -1.0, dtype=np.float32)
        gdstB = np.full((G, kB * P), -1.0, dtype=np.float32)
        geA = np.zeros((G, kA * P), dtype=np.int64)
        geB = np.zeros((G, kB * P), dtype=np.int64)
        pos = 0
        for g in range(G):
            nA, nB = int(cntA[g]), int(cntB[g])
            sA = e_src[pos : pos + nA]
            dA = dloc[pos : pos + nA]
            sB = e_src[pos + nA : pos + nA + nB] - HALF
            dB = dloc[pos + nA : pos + nA + nB]
            pos += nA + nB
            gidxA[g, :nA] = sA
            gidxB[g, :nB] = sB
            geA[g, :nA] = dA
            geB[g, :nB] = dB
            gdstA[g, :nA] = (dA - g * P).astype(np.float32)
            gdstB[g, :nB] = (dB - g * P).astype(np.float32)

        # batched layouts: per batch, A tiles of all groups then B tiles
        idxA_cols, idxB_cols, idxE_cols, dst_cols = [], [], [], []
        for gs in batches:
            nb = len(gs)
            idxA_cols.append(
                _wrap_idxs(np.concatenate([gidxA[g] for g in gs]), nb * kA)
            )
            idxB_cols.append(
                _wrap_idxs(np.concatenate([gidxB[g] for g in gs]), nb * kB)
            )
            idxE_cols.append(
                _wrap_idxs(
                    np.concatenate(
                        [geA[g] for g in gs] + [geB[g] for g in gs]
                    ),
                    nb * K,
                )
            )
            dstf_b = np.concatenate(
                [gdstA[g] for g in gs] + [gdstB[g] for g in gs]
            )  # [nb*K*P]
            dst_cols.append(np.ascontiguousarray(dstf_b.reshape(nb * K, P).T))
        idxA_sb = np.concatenate(idxA_cols, axis=1)
        idxB_sb = np.concatenate(idxB_cols, axis=1)
        idxE_sb = np.concatenate(idxE_cols, axis=1)
        dstf_sb = np.concatenate(dst_cols, axis=1).astype(BF16)

        hT = np.ascontiguousarray(h[c * NOWN : (c + 1) * NOWN, :].T).astype(BF16)

        m = {
            "hT": hT,
            "iota": np.broadcast_to(
                np.arange(P, dtype=np.float32)[None, :], (P, P)
            ).astype(BF16),
            "ident": np.eye(P, dtype=np.float32),
            "dstf": dstf_sb,
            "idxA": idxA_sb,
            "idxB": idxB_sb,
            "idxE": idxE_sb,
            "Wa1": Waug[0],
            "Wa2": Waug[1],
            "Wa3": Waug[2],
        }
        in_maps.append(m)

    cfg = dict(
        N=N,
        E=E,
        NFEAT=NFEAT,
        NOWN=NOWN,
        G=G,
        HALF=HALF,
        H=H,
        FEAT=FEAT,
        D=D,
        FO=FO,
        ROW=ROW,
        NCLASS=NCLASS,
        kA=kA,
        kB=kB,
        K=K,
    )
    return cfg, in_maps


# ----------------------------------------------------------------------------
# Bass program
# ----------------------------------------------------------------------------


def _build(cfg, mm_f32r=True):
    import concourse.bacc as bacc
    import concourse.mybir as mybir
    import concourse.tile as tile

    NOWN, G, HALF = cfg["NOWN"], cfg["G"], cfg["HALF"]
    N, NFEAT, H = cfg["N"], cfg["NFEAT"], cfg["H"]
    FEAT, FO, ROW, D = cfg["FEAT"], cfg["FO"], cfg["ROW"], cfg["D"]
    NCLASS = cfg["NCLASS"]
    kA, kB, K = cfg["kA"], cfg["kB"], cfg["K"]
    NEG = 0.2
    f32 = mybir.dt.float32
    bf16 = mybir.dt.bfloat16
    i16 = mybir.dt.int16
    AF = mybir.ActivationFunctionType
    OP = mybir.AluOpType

    F_IN = [NFEAT, FEAT[0], FEAT[1]]
    KT = [math.ceil(f / P) for f in F_IN]
    KTmax = max(KT)
    batches = _batches(G)
    NBT = sum(len(gs) for gs in batches) * K  # total slot tiles (== G*K)
    ERW = 128  # er table row: 128 bf16 elements = 256B

    nc = bacc.Bacc(
        "TRN2", target_bir_lowering=False, debug=False, num_devices=NCORES
    )

    # ---- I/O ----------------------------------------------------------------
    hT_d = nc.dram_tensor("hT", [NFEAT, NOWN], bf16, kind="ExternalInput")
    iota_d = nc.dram_tensor("iota", [P, P], bf16, kind="ExternalInput")
    ident_d = nc.dram_tensor("ident", [P, P], f32, kind="ExternalInput")
    dstf_d = nc.dram_tensor("dstf", [P, NBT], bf16, kind="ExternalInput")
    idxA_d = nc.dram_tensor("idxA", [P, G * kA * 8], i16, kind="ExternalInput")
    idxB_d = nc.dram_tensor("idxB", [P, G * kB * 8], i16, kind="ExternalInput")
    idxE_d = nc.dram_tensor("idxE", [P, G * K * 8], i16, kind="ExternalInput")
    W_d = [
        nc.dram_tensor(f"Wa{i + 1}", [F_IN[i], FO[i]], bf16, kind="ExternalInput")
        for i in range(3)
    ]
    out_d = nc.dram_tensor("out", [NOWN, NCLASS], f32, kind="ExternalOutput")

    # internal DRAM per layer
    ag_in = [
        nc.dram_tensor(f"ag_in{i}", [NOWN, ROW[i]], bf16, kind="Internal")
        for i in range(3)
    ]
    ag_out = [
        nc.dram_tensor(
            f"ag_out{i}", [NCORES * NOWN, ROW[i]], bf16, kind="Internal",
            addr_space="Shared",
        )
        for i in range(3)
    ]
    er_tab = [
        nc.dram_tensor(f"er_tab{i}", [G * P, ERW], bf16, kind="Internal")
        for i in range(3)
    ]

    rg = [list(range(NCORES))]

    with tile.TileContext(nc, num_cores=NCORES) as tc:
        with (
            tc.tile_pool(name="const", bufs=1) as cpool,
            tc.tile_pool(name="work", bufs=2) as wpool,
            tc.tile_pool(name="gath", bufs=2) as gpool,
            tc.tile_pool(name="psum", bufs=2, space="PSUM") as pspool,
        ):
            iota_t = cpool.tile([P, P], bf16, name="iota_t")
            ident_t = cpool.tile([P, P], f32, name="ident_t")
            dstf_t = cpool.tile([P, NBT], bf16, name="dstf_t")
            idxA_t = cpool.tile([P, G * kA * 8], i16, name="idxA_t")
            idxB_t = cpool.tile([P, G * kB * 8], i16, name="idxB_t")
            idxE_t = cpool.tile([P, G * K * 8], i16, name="idxE_t")
            nc.sync.dma_start(iota_t[:], iota_d[:])
            nc.sync.dma_start(ident_t[:], ident_d[:])
            nc.sync.dma_start(dstf_t[:], dstf_d[:])
            nc.sync.dma_start(idxA_t[:], idxA_d[:])
            nc.sync.dma_start(idxB_t[:], idxB_d[:])
            nc.sync.dma_start(idxE_t[:], idxE_d[:])

            W_t = []
            for l in range(3):
                slices = []
                for k in range(KT[l]):
                    r0 = k * P
                    r1 = min(r0 + P, F_IN[l])
                    w = cpool.tile([P, FO[l]], bf16, name=f"W{l}_{k}")
                    nc.sync.dma_start(w[: r1 - r0, :], W_d[l][r0:r1, :])
                    slices.append(w)
                W_t.append(slices)

            # x^T tiles, [128, NOWN] per 128-row slice of the input features
            xT = [
                cpool.tile([P, NOWN], bf16, name=f"xT{k}") for k in range(KTmax)
            ]
            for k in range(KT[0]):
                r0, r1 = k * P, min((k + 1) * P, NFEAT)
                nc.sync.dma_start(xT[k][: r1 - r0, :], hT_d[r0:r1, :])

            er_big = cpool.tile([P, G * ERW], bf16, name="er_big")

            for l in range(3):
                FT, FOL, RW, DL = FEAT[l], FO[l], ROW[l], D[l]
                last = l == 2

                # ---------------- Phase A: feat/el/er for owned nodes -------
                for g in range(G):
                    nn = min(P, NOWN - g * P)
                    psA = pspool.tile([P, FOL], f32, name="psA", tag="psA")
                    for k in range(KT[l]):
                        kk = min(P, F_IN[l] - k * P)
                        lhs = xT[k][:kk, g * P : g * P + nn]
                        rhs = W_t[l][k][:kk, :]
                        nc.tensor.matmul(
                            psA[:nn, :],
                            lhsT=lhs,
                            rhs=rhs,
                            start=(k == 0),
                            stop=(k == KT[l] - 1),
                        )
                    stage = wpool.tile([P, RW], bf16, name="stage", tag="stage")
                    nc.vector.tensor_copy(stage[:nn, 0 : FT + H], psA[:nn, 0 : FT + H])
                    if RW > FT + H:
                        nc.vector.memset(stage[:, FT + H : RW], 0.0)
                    # er for own nodes: [P, G*ERW] staging (cols 0:H used)
                    nc.vector.tensor_copy(
                        er_big[:nn, g * ERW : g * ERW + H], psA[:nn, FT + H : FOL]
                    )
                    nc.sync.dma_start(
                        ag_in[l][g * P : g * P + nn, :], stage[:nn, :]
                    )
                # er table: [128, G*ERW] -> [G*128, ERW]
                nc.sync.dma_start(
                    er_tab[l][:].rearrange("(g p) c -> p g c", p=P),
                    er_big[:].rearrange("p (g c) -> p g c", c=ERW),
                )

                # ---------------- AllGather --------------------------------
                nc.gpsimd.collective_compute(
                    "AllGather",
                    mybir.AluOpType.bypass,
                    replica_groups=rg,
                    ins=[ag_in[l][:]],
                    outs=[ag_out[l][:]],
                )

                tabA = ag_out[l][0:HALF, :]
                tabB = ag_out[l][HALF:N, :]

                # ---------------- Edge phase (batched gathers) -------------
                bt_base = 0  # first slot-tile index of this batch
                ia = ib = ie = 0  # idx column offsets
                for gs in batches:
                    nb = len(gs)
                    fb = gpool.tile([P, BATCH * K * RW], bf16, name="fb", tag="fb")
                    eb = gpool.tile([P, BATCH * K * ERW], bf16, name="eb", tag="eb")
                    f3 = fb[:].rearrange("p (k r) -> p k r", r=RW)
                    e3 = eb[:].rearrange("p (k r) -> p k r", r=ERW)
                    nc.gpsimd.dma_gather(
                        f3[:, 0 : nb * kA, :],
                        tabA,
                        idxA_t[:, ia : ia + nb * kA * 8],
                        nb * kA * P,
                        nb * kA * P,
                        RW,
                        elem_step=RW,
                        single_packet=False,
                    )
                    nc.gpsimd.dma_gather(
                        f3[:, nb * kA : nb * K, :],
                        tabB,
                        idxB_t[:, ib : ib + nb * kB * 8],
                        nb * kB * P,
                        nb * kB * P,
                        RW,
                        elem_step=RW,
                        single_packet=False,
                    )
                    nc.gpsimd.dma_gather(
                        e3[:, 0 : nb * K, :],
                        er_tab[l][:],
                        idxE_t[:, ie : ie + nb * K * 8],
                        nb * K * P,
                        nb * K * P,
                        ERW,
                        elem_step=ERW,
                        single_packet=False,
                    )
                    ia += nb * kA * 8
                    ib += nb * kB * 8
                    ie += nb * K * 8

                    for gi, g in enumerate(gs):
                        nn = min(P, NOWN - g * P)
                        a0 = gi * kA  # A-tile offset in batch
                        b0 = nb * kA + gi * kB  # B-tile offset in batch

                        # e = exp(leaky_relu(el + er)) for K tiles
                        ee = wpool.tile([P, K * H], bf16, name="ee", tag="ee")
                        ee3 = ee[:].rearrange("p (k h) -> p k h", h=H)
                        nc.vector.tensor_add(
                            ee3[:, 0:kA, :],
                            f3[:, a0 : a0 + kA, FT : FT + H],
                            e3[:, a0 : a0 + kA, 0:H],
                        )
                        nc.vector.tensor_add(
                            ee3[:, kA:K, :],
                            f3[:, b0 : b0 + kB, FT : FT + H],
                            e3[:, b0 : b0 + kB, 0:H],
                        )
                        # leaky_relu(x) = max(0.2*x, x)
                        nc.vector.scalar_tensor_tensor(
                            out=ee[:], in0=ee[:], scalar=NEG, in1=ee[:],
                            op0=OP.mult, op1=OP.max,
                        )
                        nc.scalar.activation(ee[:], ee[:], AF.Exp)

                        ps_out = pspool.tile(
                            [P, FT + H], f32, name="ps_out", tag="ps_out"
                        )
                        for t in range(K):
                            ft = a0 + t if t < kA else b0 + (t - kA)
                            dcol = bt_base + ft
                            oh = wpool.tile([P, P], bf16, name="oh", tag="oh", bufs=3)
                            nc.vector.tensor_tensor(
                                out=oh[:],
                                in0=dstf_t[:, dcol : dcol + 1].to_broadcast([P, P]),
                                in1=iota_t[:],
                                op=OP.is_equal,
                            )
                            fs = wpool.tile(
                                [P, FT + H], bf16, name="fs", tag="fs", bufs=3
                            )
                            nc.vector.tensor_mul(
                                fs[:, 0:FT].rearrange("p (h d) -> p h d", h=H),
                                f3[:, ft, 0:FT].rearrange("p (h d) -> p h d", h=H),
                                ee[:, t * H : (t + 1) * H].to_broadcast([P, H, DL]),
                            )
                            nc.vector.tensor_copy(
                                fs[:, FT : FT + H], ee[:, t * H : (t + 1) * H]
                            )
                            nc.tensor.matmul(
                                ps_out[:],
                                lhsT=oh[:],
                                rhs=fs[:],
                                start=(t == 0),
                                stop=(t == K - 1),
                            )

                        s_r = wpool.tile([P, H], f32, name="s_r", tag="s_r")
                        nc.vector.tensor_scalar_max(
                            s_r[:], ps_out[:, FT : FT + H], 1e-30
                        )
                        nc.vector.reciprocal(s_r[:], s_r[:])
                        if last:
                            nc.vector.tensor_scalar_mul(s_r[:], s_r[:], 1.0 / H)
                        xg = wpool.tile([P, FT], f32, name="xg", tag="xg")
                        nc.vector.tensor_mul(
                            xg[:].rearrange("p (h d) -> p h d", h=H),
                            ps_out[:, 0:FT].rearrange("p (h d) -> p h d", h=H),
                            s_r[:].to_broadcast([P, H, DL]),
                        )

                        if not last:
                            # elu(x) = max(x, exp(min(x, 0)) - 1), then transpose
                            mg = wpool.tile([P, FT], f32, name="mg", tag="mg")
                            nc.vector.tensor_scalar_min(mg[:], xg[:], 0.0)
                            nc.scalar.activation(mg[:], mg[:], AF.Exp)
                            nc.vector.scalar_tensor_tensor(
                                out=xg[:],
                                in0=mg[:],
                                scalar=-1.0,
                                in1=xg[:],
                                op0=OP.add,
                                op1=OP.max,
                            )
                            for kk in range(KT[l + 1]):
                                c0 = kk * P
                                c1 = min(c0 + P, FT)
                                w = c1 - c0
                                pt = pspool.tile([P, P], f32, name="pt", tag="pt")
                                nc.tensor.transpose(
                                    pt[:w, :], xg[:, c0:c1], ident_t[:]
                                )
                                nc.vector.tensor_copy(
                                    xT[kk][:w, g * P : g * P + nn], pt[:w, :nn]
                                )
                        else:
                            # mean over heads -> [nn, NCLASS] -> DRAM
                            o1 = wpool.tile([P, NCLASS], f32, name="o1", tag="o1")
                            o2 = wpool.tile([P, NCLASS], f32, name="o2", tag="o2")
                            nc.vector.tensor_add(
                                o1[:], xg[:, 0:NCLASS], xg[:, NCLASS : 2 * NCLASS]
                            )
                            nc.vector.tensor_add(
                                o2[:],
                                xg[:, 2 * NCLASS : 3 * NCLASS],
                                xg[:, 3 * NCLASS : 4 * NCLASS],
                            )
                            nc.vector.tensor_add(o1[:], o1[:], o2[:])
                            nc.sync.dma_start(
                                out_d[g * P : g * P + nn, :], o1[:nn, :]
                            )
                    bt_base += nb * K

    nc.compile()
    return nc


# ----------------------------------------------------------------------------
# Driver
# ----------------------------------------------------------------------------

_CACHE = {}


def _get_nc(cfg, mm_f32r=True):
    key = str(sorted(cfg.items())) + str(mm_f32r)
    if key not in _CACHE:
        _CACHE[key] = _build(cfg, mm_f32r=mm_f32r)
    return _CACHE[key]


def _run(inputs, trace=False, mm_f32r=True, use_sim=False, bench_iters=0):
    cfg, in_maps = _prepare(inputs)
    nc = _get_nc(cfg, mm_f32r)

    if use_sim:
        from concourse.bass_interp import MultiCoreSim

        sim = MultiCoreSim(nc, num_cores=NCORES, require_finite=False)
        for c in range(NCORES):
            for k, v in in_maps[c].items():
                sim.cores[c].tensor(k)[:] = v
        sim.simulate(check_with_hw=False)
        outs = [np.array(sim.cores[c].tensor("out")) for c in range(NCORES)]
        res = None
    else:
        outs, res = _pjrt_run(nc, in_maps, bench_iters=bench_iters)

    out = np.concatenate(outs, axis=0).astype(np.float32)
    return out, res


def _pjrt_run(nc, in_maps, bench_iters=0):
    """Execute the SPMD program on the 8 axon-tunneled cores via PJRT.

    Mirrors concourse.bass2jax.run_bass_via_pjrt but keeps the compiled
    executable so warm re-runs can be timed (bench_iters > 0)."""
    import time as _time

    import jax
    import numpy as _np
    from jax.sharding import Mesh, PartitionSpec
    from jax.experimental.shard_map import shard_map

    import concourse.mybir as mybir
    from concourse.bass2jax import (
        _bass_exec_p,
        install_neuronx_cc_hook,
        partition_id_tensor,
    )

    install_neuronx_cc_hook()
    n_cores = len(in_maps)

    partition_name = nc.partition_id_tensor.name if nc.partition_id_tensor else None
    in_names, out_names, out_avals, zero_outs = [], [], [], []
    for alloc in nc.m.functions[0].allocations:
        if not isinstance(alloc, mybir.MemoryLocationSet):
            continue
        name = alloc.memorylocations[0].name
        if alloc.kind == "ExternalInput":
            if name != partition_name:
                in_names.append(name)
        elif alloc.kind == "ExternalOutput":
            shape = tuple(alloc.tensor_shape)
            dtype = mybir.dt.np(alloc.dtype)
            out_names.append(name)
            out_avals.append(jax.core.ShapedArray(shape, dtype))
            zero_outs.append(_np.zeros(shape, dtype))
    n_params = len(in_names)
    n_outs = len(out_avals)
    in_names_all = list(in_names) + list(out_names)
    if partition_name is not None:
        in_names_all.append(partition_name)
    donate = tuple(range(n_params, n_params + n_outs))

    def _body(*args):
        operands = list(args)
        if partition_name is not None:
            operands.append(partition_id_tensor())
        outs = _bass_exec_p.bind(
            *operands,
            out_avals=tuple(out_avals),
            in_names=tuple(in_names_all),
            out_names=tuple(out_names),
            lowering_input_output_aliases=(),
            sim_require_finite=True,
            sim_require_nnan=True,
            nc=nc,
        )
        return tuple(outs)

    devices = jax.devices()[:n_cores]
    mesh = Mesh(_np.asarray(devices), ("core",))
    in_specs = (PartitionSpec("core"),) * (n_params + n_outs)
    out_specs = (PartitionSpec("core"),) * n_outs
    sharded = jax.jit(
        shard_map(
            _body, mesh=mesh, in_specs=in_specs, out_specs=out_specs,
            check_rep=False,
        ),
        donate_argnums=donate,
        keep_unused=True,
    )
    concat_in = [
        _np.concatenate([_np.asarray(in_maps[c][nm]) for c in range(n_cores)], axis=0)
        for nm in in_names
    ]

    def _zeros_dev():
        return [
            jax.device_put(
                _np.zeros((n_cores * z.shape[0], *z.shape[1:]), z.dtype),
                jax.sharding.NamedSharding(mesh, PartitionSpec("core")),
            )
            for z in zero_outs
        ]

    dev_in = [
        jax.device_put(a, jax.sharding.NamedSharding(mesh, PartitionSpec("core")))
        for a in concat_in
    ]

    out_arrs = sharded(*dev_in, *_zeros_dev())
    jax.block_until_ready(out_arrs)

    times = []
    for _ in range(bench_iters):
        zs = _zeros_dev()
        jax.block_until_ready(zs)
        t0 = _time.perf_counter()
        o = sharded(*dev_in, *zs)
        jax.block_until_ready(o)
        times.append(_time.perf_counter() - t0)

    outs = [
        {
            nm: _np.asarray(out_arrs[i]).reshape(n_cores, *out_avals[i].shape)[c]
            for i, nm in enumerate(out_names)
        }
        for c in range(n_cores)
    ]
    res = {"times_s": times, "min_time_ns": int(min(times) * 1e9) if times else None}
    return [o["out"] for o in outs], res


def kernel(**inputs):
    out, _ = _run(inputs, trace=False)
    return out


# revision 9
# speedup vs baseline: 1.0607x; 1.0607x over previous
"""GAT (3-layer, PPI-style) Bass/Tile kernel for 8 Trainium2 NeuronCores.

Strategy (graph/data parallel, dst-ownership sharding):
  - Nodes are sharded contiguously: core c owns nodes [c*NOWN, (c+1)*NOWN).
  - Edges live on the core owning dst; per core they are grouped by
    128-node dst groups and sorted so that edge-softmax segment reductions
    become dense one-hot matmuls on the tensor engine.
  - Per layer: Phase A computes feat/el/er for owned nodes with one matmul
    against W_aug = [W | W@al_bd | W@ar_bd]; an AllGather publishes bf16
    rows [(feat_h | 1.0) x H | el] to every core; SWDGE dma_gather pulls
    the per-edge rows by src (trailing -1 padding is trimmed by the Q7
    ucode); per-slot er comes from a tiny PE matmul against transposed
    one-hot tiles streamed from HBM (no per-edge er gather); one fused
    one-hot matmul per slot tile accumulates both sum_e ex_e * feat_src
    and sum_e ex_e (via the interleaved 1.0 columns); node-level
    normalization, ELU and a PE transpose produce the next layer's x^T.

All graph-dependent index structures are computed on the host inside
kernel() and shipped as tensor inputs, so one SPMD program serves all
8 cores.
"""

import math

import ml_dtypes
import numpy as np

BF16 = ml_dtypes.bfloat16
P = 128
NCORES = 8


# ----------------------------------------------------------------------------
# Host-side preparation
# ----------------------------------------------------------------------------


def _wrap_idxs(idx, k):
    """int16 index array for dma_gather: wrapped in 16 partitions, replicated
    8x across the 128 partitions. idx: [k*128] -> [128, k*8]."""
    assert idx.shape[0] == k * P
    w = idx.astype(np.int16).reshape(k * 8, 16).T  # [16, k*8]
    return np.ascontiguousarray(np.tile(w, (8, 1)))  # [128, k*8]


def _prepare(inputs):
    h = np.asarray(inputs["h"], dtype=np.float32)
    src = np.asarray(inputs["src"]).astype(np.int64)
    dst = np.asarray(inputs["dst"]).astype(np.int64)

    N, NFEAT = h.shape
    E = src.shape[0]
    assert N % NCORES == 0
    NOWN = N // NCORES
    G = math.ceil(NOWN / P)
    HALF = (N + 1) // 2
    assert HALF <= 32767, "table half exceeds int16 gather index range"
    assert NOWN <= 32767

    Ws, als, ars = [], [], []
    for i in (1, 2, 3):
        Ws.append(np.asarray(inputs[f"W{i}"], dtype=np.float32))
        als.append(np.asarray(inputs[f"al{i}"], dtype=np.float32))
        ars.append(np.asarray(inputs[f"ar{i}"], dtype=np.float32))
    H = als[0].shape[0]
    FEAT = [W.shape[1] for W in Ws]  # H*D per layer
    D = [f // H for f in FEAT]
    NCLASS = D[-1]

    # W_aug = [W | W @ al_bd | W @ ar_bd] with al_bd[h*D+d, h] = al[h, d]
    Waug = []
    for W, al, ar, f, d in zip(Ws, als, ars, FEAT, D):
        al_bd = np.zeros((f, H), dtype=np.float32)
        ar_bd = np.zeros((f, H), dtype=np.float32)
        for hh in range(H):
            al_bd[hh * d : (hh + 1) * d, hh] = al[hh]
            ar_bd[hh * d : (hh + 1) * d, hh] = ar[hh]
        Waug.append(
            np.concatenate([W, W @ al_bd, W @ ar_bd], axis=1).astype(BF16)
        )
    FO = [f + 2 * H for f in FEAT]
    # fs width per layer: (feat | 1.0) interleaved per head
    FS = [H * (d + 1) for d in D]
    # bf16 gather-table row widths in elements (bytes multiple of 256):
    # [(feat_h | 1.0) x H | el]
    ROW = [math.ceil((fs + H) * 2 / 256) * 128 for fs in FS]

    # ---- edge partitioning --------------------------------------------------
    owner = dst // NOWN
    per_core = []
    maxA = maxB = 0
    for c in range(NCORES):
        sel = np.nonzero(owner == c)[0]
        e_src = src[sel]
        e_dst = dst[sel]
        dloc = e_dst - c * NOWN  # 0..NOWN-1
        grp = dloc // P  # dst group
        half = (e_src >= HALF).astype(np.int64)
        order = np.lexsort((e_src, half, grp))
        e_src, dloc, grp, half = e_src[order], dloc[order], grp[order], half[order]
        cntA = np.zeros(G, dtype=np.int64)
        cntB = np.zeros(G, dtype=np.int64)
        for g in range(G):
            m = grp == g
            cntA[g] = int(np.count_nonzero(m & (half == 0)))
            cntB[g] = int(np.count_nonzero(m & (half == 1)))
        per_core.append((e_src, dloc, grp, half, cntA, cntB))
        maxA = max(maxA, int(cntA.max()) if G else 0)
        maxB = max(maxB, int(cntB.max()) if G else 0)

    kA = max(1, math.ceil(maxA / P))
    kB = max(1, math.ceil(maxB / P))
    K = kA + kB

    in_maps = []
    for c in range(NCORES):
        e_src, dloc, grp, half, cntA, cntB = per_core[c]
        idxA = np.zeros((G, kA * P), dtype=np.int64)
        idxB = np.zeros((G, kB * P), dtype=np.int64)
        dstf = np.full((G, K * P), -1.0, dtype=np.float32)
        pos = 0
        for g in range(G):
            nA, nB = int(cntA[g]), int(cntB[g])
            sA = e_src[pos : pos + nA]
            dA = dloc[pos : pos + nA]
            sB = e_src[pos + nA : pos + nA + nB] - HALF
            dB = dloc[pos + nA : pos + nA + nB]
            pos += nA + nB
            idxA[g, :nA] = sA
            idxB[g, :nB] = sB
            dstf[g, :nA] = (dA - g * P).astype(np.float32)
            dstf[g, kA * P : kA * P + nB] = (dB - g * P).astype(np.float32)

        idxA_sb = np.concatenate([_wrap_idxs(idxA[g], kA) for g in range(G)], axis=1)
        idxB_sb = np.concatenate([_wrap_idxs(idxB[g], kB) for g in range(G)], axis=1)
        # dstf as SBUF layout [128, G*K]: [p, g*K+t] = dst_local of slot t*128+p
        dstf_sb = np.ascontiguousarray(dstf.reshape(G * K, P).T).astype(BF16)
        # transposed one-hot table [128 nodes, G*K*128 slots]:
        # ohT[p, (g*K+t)*128+j] = 1 if dstf[g, t*128+j] == p
        flat = dstf.reshape(-1)  # [G*K*P]
        ohT = (flat[None, :] == np.arange(P, dtype=np.float32)[:, None]).astype(BF16)

        hT = np.ascontiguousarray(h[c * NOWN : (c + 1) * NOWN, :].T).astype(BF16)
        iota_k = np.tile(
            np.arange(P, dtype=np.float32)[None, :], (P, K)
        ).astype(BF16)  # [128, K*128]
        m = {
            "hT": hT,
            "iotaK": iota_k,
            "ident": np.eye(P, dtype=np.float32),
            "dstf": dstf_sb,
            "ohT": np.ascontiguousarray(ohT),
            "idxA": idxA_sb,
            "idxB": idxB_sb,
            "Wa1": Waug[0],
            "Wa2": Waug[1],
            "Wa3": Waug[2],
        }
        in_maps.append(m)

    cfg = dict(
        N=N,
        E=E,
        NFEAT=NFEAT,
        NOWN=NOWN,
        G=G,
        HALF=HALF,
        H=H,
        FEAT=FEAT,
        D=D,
        FO=FO,
        FS=FS,
        ROW=ROW,
        NCLASS=NCLASS,
        kA=kA,
        kB=kB,
        K=K,
    )
    return cfg, in_maps


# ----------------------------------------------------------------------------
# Bass program
# ----------------------------------------------------------------------------


def _build(cfg, mm_f32r=True):
    import concourse.bacc as bacc
    import concourse.mybir as mybir
    import concourse.tile as tile

    NOWN, G, HALF = cfg["NOWN"], cfg["G"], cfg["HALF"]
    N, NFEAT, H = cfg["N"], cfg["NFEAT"], cfg["H"]
    FEAT, FO, ROW, D = cfg["FEAT"], cfg["FO"], cfg["ROW"], cfg["D"]
    FS = cfg["FS"]
    NCLASS = cfg["NCLASS"]
    kA, kB, K = cfg["kA"], cfg["kB"], cfg["K"]
    NEG = 0.2
    f32 = mybir.dt.float32
    bf16 = mybir.dt.bfloat16
    i16 = mybir.dt.int16
    AF = mybir.ActivationFunctionType
    OP = mybir.AluOpType

    F_IN = [NFEAT, FEAT[0], FEAT[1]]
    KT = [math.ceil(f / P) for f in F_IN]
    KTmax = max(KT)

    nc = bacc.Bacc(
        "TRN2", target_bir_lowering=False, debug=False, num_devices=NCORES
    )

    # ---- I/O ----------------------------------------------------------------
    hT_d = nc.dram_tensor("hT", [NFEAT, NOWN], bf16, kind="ExternalInput")
    iotaK_d = nc.dram_tensor("iotaK", [P, K * P], bf16, kind="ExternalInput")
    ident_d = nc.dram_tensor("ident", [P, P], f32, kind="ExternalInput")
    dstf_d = nc.dram_tensor("dstf", [P, G * K], bf16, kind="ExternalInput")
    ohT_d = nc.dram_tensor("ohT", [P, G * K * P], bf16, kind="ExternalInput")
    idxA_d = nc.dram_tensor("idxA", [P, G * kA * 8], i16, kind="ExternalInput")
    idxB_d = nc.dram_tensor("idxB", [P, G * kB * 8], i16, kind="ExternalInput")
    W_d = [
        nc.dram_tensor(f"Wa{i + 1}", [F_IN[i], FO[i]], bf16, kind="ExternalInput")
        for i in range(3)
    ]
    out_d = nc.dram_tensor("out", [NOWN, NCLASS], f32, kind="ExternalOutput")

    # internal DRAM per layer
    ag_in = [
        nc.dram_tensor(f"ag_in{i}", [NOWN, ROW[i]], bf16, kind="Internal")
        for i in range(3)
    ]
    ag_out = [
        nc.dram_tensor(
            f"ag_out{i}", [NCORES * NOWN, ROW[i]], bf16, kind="Internal",
            addr_space="Shared",
        )
        for i in range(3)
    ]

    rg = [list(range(NCORES))]

    with tile.TileContext(nc, num_cores=NCORES) as tc:
        with (
            tc.tile_pool(name="const", bufs=1) as cpool,
            tc.tile_pool(name="work", bufs=2) as wpool,
            tc.tile_pool(name="gath", bufs=2) as gpool,
            tc.tile_pool(name="psum", bufs=2, space="PSUM") as pspool,
        ):
            iotaK_t = cpool.tile([P, K * P], bf16, name="iotaK_t")
            ident_t = cpool.tile([P, P], f32, name="ident_t")
            dstf_t = cpool.tile([P, G * K], bf16, name="dstf_t")
            idxA_t = cpool.tile([P, G * kA * 8], i16, name="idxA_t")
            idxB_t = cpool.tile([P, G * kB * 8], i16, name="idxB_t")
            nc.sync.dma_start(iotaK_t[:], iotaK_d[:])
            nc.sync.dma_start(ident_t[:], ident_d[:])
            nc.sync.dma_start(dstf_t[:], dstf_d[:])
            nc.sync.dma_start(idxA_t[:], idxA_d[:])
            nc.sync.dma_start(idxB_t[:], idxB_d[:])

            W_t = []
            for l in range(3):
                slices = []
                for k in range(KT[l]):
                    r0 = k * P
                    r1 = min(r0 + P, F_IN[l])
                    w = cpool.tile([P, FO[l]], bf16, name=f"W{l}_{k}")
                    nc.sync.dma_start(w[: r1 - r0, :], W_d[l][r0:r1, :])
                    slices.append(w)
                W_t.append(slices)

            # x^T tiles, [128, NOWN] per 128-row slice of the input features
            xT = [
                cpool.tile([P, NOWN], bf16, name=f"xT{k}") for k in range(KTmax)
            ]
            for k in range(KT[0]):
                r0, r1 = k * P, min((k + 1) * P, NFEAT)
                nc.sync.dma_start(xT[k][: r1 - r0, :], hT_d[r0:r1, :])

            # er for own nodes, kept on-chip: [128, G*H] bf16
            er_big = cpool.tile([P, G * H], bf16, name="er_big")

            for l in range(3):
                FT, FOL, RW, DL, FSL = FEAT[l], FO[l], ROW[l], D[l], FS[l]
                last = l == 2

                # ---------------- Phase A: feat/el/er for owned nodes -------
                for g in range(G):
                    nn = min(P, NOWN - g * P)
                    psA = pspool.tile([P, FOL], f32, name="psA", tag="psA")
                    for k in range(KT[l]):
                        kk = min(P, F_IN[l] - k * P)
                        lhs = xT[k][:kk, g * P : g * P + nn]
                        rhs = W_t[l][k][:kk, :]
                        nc.tensor.matmul(
                            psA[:nn, :],
                            lhsT=lhs,
                            rhs=rhs,
                            start=(k == 0),
                            stop=(k == KT[l] - 1),
                        )
                    stage = wpool.tile([P, RW], bf16, name="stage", tag="stage")
                    st3 = stage[:, 0:FSL].rearrange("p (h e) -> p h e", h=H)
                    nc.vector.tensor_copy(
                        st3[:, :, 0:DL],
                        psA[:, 0:FT].rearrange("p (h d) -> p h d", h=H),
                    )
                    nc.vector.memset(st3[:, :, DL : DL + 1], 1.0)
                    nc.vector.tensor_copy(
                        stage[:, FSL : FSL + H], psA[:, FT : FT + H]
                    )
                    if RW > FSL + H:
                        nc.vector.memset(stage[:, FSL + H : RW], 0.0)
                    nc.vector.tensor_copy(
                        er_big[:, g * H : g * H + H], psA[:, FT + H : FOL]
                    )
                    nc.sync.dma_start(
                        ag_in[l][g * P : g * P + nn, :], stage[:nn, :]
                    )

                # ---------------- AllGather --------------------------------
                nc.gpsimd.collective_compute(
                    "AllGather",
                    mybir.AluOpType.bypass,
                    replica_groups=rg,
                    ins=[ag_in[l][:]],
                    outs=[ag_out[l][:]],
                )

                tabA = ag_out[l][0:HALF, :]
                tabB = ag_out[l][HALF:N, :]

                # ---------------- Edge phase -------------------------------
                for g in range(G):
                    nn = min(P, NOWN - g * P)
                    fb = gpool.tile([P, K * RW], bf16, name="fb", tag="fb")
                    f3 = fb[:].rearrange("p (k r) -> p k r", r=RW)
                    if g < 2:
                        # stale-data guard for pad slots: the first use of
                        # each pool buffer per layer may hold NaN garbage
                        nc.vector.memset(fb[:], 0.0)
                    oht = gpool.tile([P, K * P], bf16, name="oht", tag="oht")
                    nc.sync.dma_start(
                        oht[:], ohT_d[:, g * K * P : (g + 1) * K * P]
                    )
                    nc.gpsimd.dma_gather(
                        f3[:, 0:kA, :],
                        tabA,
                        idxA_t[:, g * kA * 8 : (g + 1) * kA * 8],
                        kA * P,
                        kA * P,
                        RW,
                        elem_step=RW,
                    )
                    nc.gpsimd.dma_gather(
                        f3[:, kA:K, :],
                        tabB,
                        idxB_t[:, g * kB * 8 : (g + 1) * kB * 8],
                        kB * P,
                        kB * P,
                        RW,
                        elem_step=RW,
                    )

                    # per-slot er via transposed one-hot matmuls
                    er_ps = pspool.tile([P, K * H], f32, name="er_ps", tag="er_ps")
                    for t in range(K):
                        nc.tensor.matmul(
                            er_ps[:, t * H : (t + 1) * H],
                            lhsT=oht[:, t * P : (t + 1) * P],
                            rhs=er_big[:, g * H : (g + 1) * H],
                            start=True,
                            stop=True,
                        )

                    # e = exp(leaky_relu(el + er)) for all K tiles
                    ee = wpool.tile([P, K * H], bf16, name="ee", tag="ee")
                    nc.vector.tensor_add(
                        ee[:].rearrange("p (k h) -> p k h", h=H),
                        f3[:, :, FSL : FSL + H],
                        er_ps[:].rearrange("p (k h) -> p k h", h=H),
                    )
                    nc.vector.scalar_tensor_tensor(
                        out=ee[:], in0=ee[:], scalar=NEG, in1=ee[:],
                        op0=OP.mult, op1=OP.max,
                    )
                    nc.scalar.activation(ee[:], ee[:], AF.Exp)

                    # one-hot tiles for all K slots in one op
                    oh = wpool.tile([P, K * P], bf16, name="oh", tag="oh")
                    nc.vector.tensor_tensor(
                        out=oh[:].rearrange("p (k q) -> p k q", q=P),
                        in0=dstf_t[:, g * K : (g + 1) * K]
                        .rearrange("p k -> p k ()")
                        .to_broadcast([P, K, P]),
                        in1=iotaK_t[:].rearrange("p (k q) -> p k q", q=P),
                        op=OP.is_equal,
                    )

                    # fs = row * ee (the interleaved 1.0 columns produce ee)
                    fsb = wpool.tile([P, K * FSL], bf16, name="fsb", tag="fsb")
                    nc.vector.tensor_mul(
                        fsb[:].rearrange("p (k h e) -> p k h e", k=K, h=H),
                        f3[:, :, 0:FSL].rearrange("p k (h e) -> p k h e", h=H),
                        ee[:]
                        .rearrange("p (k h) -> p k h ()", h=H)
                        .to_broadcast([P, K, H, DL + 1]),
                    )

                    ps_out = pspool.tile([P, FSL], f32, name="ps_out", tag="ps_out")
                    for t in range(K):
                        nc.tensor.matmul(
                            ps_out[:],
                            lhsT=oh[:, t * P : (t + 1) * P],
                            rhs=fsb[:, t * FSL : (t + 1) * FSL],
                            start=(t == 0),
                            stop=(t == K - 1),
                        )

                    po3 = ps_out[:].rearrange("p (h e) -> p h e", h=H)
                    s_r = wpool.tile([P, H], f32, name="s_r", tag="s_r")
                    nc.vector.tensor_scalar_max(
                        s_r[:], po3[:, :, DL : DL + 1].rearrange("p h e -> p (h e)"),
                        1e-30,
                    )
                    nc.vector.reciprocal(s_r[:], s_r[:])
                    if last:
                        nc.vector.tensor_scalar_mul(s_r[:], s_r[:], 1.0 / H)
                    xg = wpool.tile([P, FT], f32, name="xg", tag="xg")
                    nc.vector.tensor_mul(
                        xg[:].rearrange("p (h d) -> p h d", h=H),
                        po3[:, :, 0:DL],
                        s_r[:].rearrange("p h -> p h ()").to_broadcast([P, H, DL]),
                    )

                    if not last:
                        # elu(x) = max(x, exp(min(x, 0)) - 1), then transpose
                        mg = wpool.tile([P, FT], f32, name="mg", tag="mg")
                        nc.vector.tensor_scalar_min(mg[:], xg[:], 0.0)
                        nc.scalar.activation(mg[:], mg[:], AF.Exp)
                        nc.vector.scalar_tensor_tensor(
                            out=xg[:],
                            in0=mg[:],
                            scalar=-1.0,
                            in1=xg[:],
                            op0=OP.add,
                            op1=OP.max,
                        )
                        for kk in range(KT[l + 1]):
                            c0 = kk * P
                            c1 = min(c0 + P, FT)
                            w = c1 - c0
                            pt = pspool.tile([P, P], f32, name="pt", tag="pt")
                            nc.tensor.transpose(
                                pt[:w, :], xg[:, c0:c1], ident_t[:]
                            )
                            nc.vector.tensor_copy(
                                xT[kk][:w, g * P : g * P + nn], pt[:w, :nn]
                            )
                    else:
                        # mean over heads -> [nn, NCLASS] -> DRAM
                        o1 = wpool.tile([P, NCLASS], f32, name="o1", tag="o1")
                        o2 = wpool.tile([P, NCLASS], f32, name="o2", tag="o2")
                        nc.vector.tensor_add(
                            o1[:], xg[:, 0:NCLASS], xg[:, NCLASS : 2 * NCLASS]
                        )
                        nc.vector.tensor_add(
                            o2[:],
                            xg[:, 2 * NCLASS : 3 * NCLASS],
                            xg[:, 3 * NCLASS : 4 * NCLASS],
                        )
                        nc.vector.tensor_add(o1[:], o1[:], o2[:])
                        nc.sync.dma_start(
                            out_d[g * P : g * P + nn, :], o1[:nn, :]
                        )

    nc.compile()
    return nc


# ----------------------------------------------------------------------------
# Driver
# ----------------------------------------------------------------------------

_CACHE = {}


def _get_nc(cfg, mm_f32r=True):
    key = str(sorted(cfg.items())) + str(mm_f32r)
    if key not in _CACHE:
        _CACHE[key] = _build(cfg, mm_f32r=mm_f32r)
    return _CACHE[key]


def _run(inputs, trace=False, mm_f32r=True, use_sim=False, bench_iters=0):
    cfg, in_maps = _prepare(inputs)
    nc = _get_nc(cfg, mm_f32r)

    if use_sim:
        from concourse.bass_interp import MultiCoreSim

        sim = MultiCoreSim(nc, num_cores=NCORES, require_finite=False)
        for c in range(NCORES):
            for k, v in in_maps[c].items():
                sim.cores[c].tensor(k)[:] = v
        sim.simulate(check_with_hw=False)
        outs = [np.array(sim.cores[c].tensor("out")) for c in range(NCORES)]
        res = None
    else:
        outs, res = _pjrt_run(nc, in_maps, bench_iters=bench_iters)

    out = np.concatenate(outs, axis=0).astype(np.float32)
    return out, res


def _pjrt_run(nc, in_maps, bench_iters=0):
    """Execute the SPMD program on the 8 axon-tunneled cores via PJRT.

    Mirrors concourse.bass2jax.run_bass_via_pjrt but keeps the compiled
    executable so warm re-runs can be timed (bench_iters > 0)."""
    import time as _time

    import jax
    import numpy as _np
    from jax.sharding import Mesh, PartitionSpec
    from jax.experimental.shard_map import shard_map

    import concourse.mybir as mybir
    from concourse.bass2jax import (
        _bass_exec_p,
        install_neuronx_cc_hook,
        partition_id_tensor,
    )

    install_neuronx_cc_hook()
    n_cores = len(in_maps)

    partition_name = nc.partition_id_tensor.name if nc.partition_id_tensor else None
    in_names, out_names, out_avals, zero_outs = [], [], [], []
    for alloc in nc.m.functions[0].allocations:
        if not isinstance(alloc, mybir.MemoryLocationSet):
            continue
        name = alloc.memorylocations[0].name
        if alloc.kind == "ExternalInput":
            if name != partition_name:
                in_names.append(name)
        elif alloc.kind == "ExternalOutput":
            shape = tuple(alloc.tensor_shape)
            dtype = mybir.dt.np(alloc.dtype)
            out_names.append(name)
            out_avals.append(jax.core.ShapedArray(shape, dtype))
            zero_outs.append(_np.zeros(shape, dtype))
    n_params = len(in_names)
    n_outs = len(out_avals)
    in_names_all = list(in_names) + list(out_names)
    if partition_name is not None:
        in_names_all.append(partition_name)
    donate = tuple(range(n_params, n_params + n_outs))

    def _body(*args):
        operands = list(args)
        if partition_name is not None:
            operands.append(partition_id_tensor())
        outs = _bass_exec_p.bind(
            *operands,
            out_avals=tuple(out_avals),
            in_names=tuple(in_names_all),
            out_names=tuple(out_names),
            lowering_input_output_aliases=(),
            sim_require_finite=True,
            sim_require_nnan=True,
            nc=nc,
        )
        return tuple(outs)

    devices = jax.devices()[:n_cores]
    mesh = Mesh(_np.asarray(devices), ("core",))
    in_specs = (PartitionSpec("core"),) * (n_params + n_outs)
    out_specs = (PartitionSpec("core"),) * n_outs
    sharded = jax.jit(
        shard_map(
            _body, mesh=mesh, in_specs=in_specs, out_specs=out_specs,
            check_rep=False,
        ),
        donate_argnums=donate,
        keep_unused=True,
    )
    concat_in = [
        _np.concatenate([_np.asarray(in_maps[c][nm]) for c in range(n_cores)], axis=0)
        for nm in in_names
    ]

    def _zeros_dev():
        return [
            jax.device_put(
                _np.zeros((n_cores * z.shape[0], *z.shape[1:]), z.dtype),
                jax.sharding.NamedSharding(mesh, PartitionSpec("core")),
            )
            for z in zero_outs
        ]

    dev_in = [
        jax.device_put(a, jax.sharding.NamedSharding(mesh, PartitionSpec("core")))
        for a in concat_in
    ]

    out_arrs = sharded(*dev_in, *_zeros_dev())
    jax.block_until_ready(out_arrs)

    times = []
    for _ in range(bench_iters):
        zs = _zeros_dev()
        jax.block_until_ready(zs)
        t0 = _time.perf_counter()
        o = sharded(*dev_in, *zs)
        jax.block_until_ready(o)
        times.append(_time.perf_counter() - t0)

    outs = [
        {
            nm: _np.asarray(out_arrs[i]).reshape(n_cores, *out_avals[i].shape)[c]
            for i, nm in enumerate(out_names)
        }
        for c in range(n_cores)
    ]
    res = {"times_s": times, "min_time_ns": int(min(times) * 1e9) if times else None}
    return [o["out"] for o in outs], res


def kernel(**inputs):
    out, _ = _run(inputs, trace=False)
    return out
